# revision 50
# baseline (speedup 1.0000x reference)
"""Trainium2 Bass kernel for ConvScoreSSREM loss (fp16-staged pipeline, v3).

Computes, for B=16384 rows (data-parallel, 2048 rows per NeuronCore x 8):
    cm        = contexts @ mat_M                    [B, E]
    scores_k  = sum_e cm[b,e] * res_k[b,e]          k in 0..4
    out[b]    = log_softmax(scores)[:, 0]

The kernel is HBM-streaming-bound.  Host-side staging transforms:
  - all streamed tensors (and mat_M) are cast to fp16 on the host before
    upload: HBM traffic halves (48MB -> 24MB per core per pass), and the
    DVE score reductions run in 16-bit 2x mode.  fp16 quantization gives
    3.6e-4 rel err on the output (gate is 2e-2); fp8 anywhere breaks the
    gate (measured 2.2e-2+ on the actual seeded inputs).
  - contexts is pre-transposed AND group-blocked on the host to
    [NG, P, KC, GB], so the PE consumes it as the stationary operand
    directly (no PE transposes) and each group's ctx DMA is one
    contiguous 1MB DRAM block with 8KB partition runs.
  - the five res tensors are interleaved into one [NG, P, GA, NK, E]
    DRAM tensor: each 512-row group loads as contiguous 1.25MB DMAs
    with 20KB partition runs.

v4 targets the single-pass critical path (what a one-shot kernel run
pays) on top of the v2 steady-state structure:
  - DGE rings are dedicated: ctxT b-splits on sync, res a-splits on
    scalar/gpsimd alternating, mat_M (two per-k-chunk DMAs) on gpsimd -
    the first matmuls/scores start after ~3MB lands, not ~6MB+.
  - the log-softmax tail runs once at pass end into an SBUF staging tile
    with a single out DMA.  (A per-group out DMA is a trap: its
    semaphore wait holds the DGE ring FIFO at the sequencer and stalls
    every load queued behind it - measured 2x steady-state regression.)

Per-core steady state (2048 rows, E=1024): per 128-row tile, 16 matmuls
(stationary = ctxT chunk fp16, moving = mat_M 512-wide halves, one PSUM
bank each) accumulate cm[128,1024] fp32; ACT copies cm to SBUF with an
fp16 downcast; 5 DVE scalar_tensor_tensor ops (fp16 in, fp32 accum)
produce the scores.

Measured (axon, r41 wall differential): ~33-42us steady-state per pass
vs ~109us for the fp32 baseline; ~33us == 24MB / 716 GB/s, the
per-device HBM roofline.
"""

import numpy as np

import concourse.bacc as bacc
import concourse.mybir as mybir
import concourse.tile as tile
from concourse.bass_utils import run_bass_kernel_spmd

B = 16384
E = 1024
NCORES = 8
BS = B // NCORES  # 2048 rows per core
P = 128
NT = BS // P      # 16 row-tiles per core
KC = E // P       # 8 contraction chunks
NK = 5            # number of res tensors
NHALF = 512       # matmul moving free-dim (one PSUM bank of fp32)
GA = 4            # row-tiles per res DMA group (512 rows)
NG = NT // GA     # 4 groups per pass

F32 = mybir.dt.float32
F16 = mybir.dt.float16

RES_NAMES = ["res0", "res1", "res2", "res3", "res4"]


DEFAULT_OPTS = dict(
    res_bufs=3,       # whole-group [P,GA,NK,E] tiles in flight (40KB each)
    ctx_bufs=5,       # per-group [P,KC,GB] ctx tiles in flight (8KB each)
    cms_bufs=3,
    pcm_bufs=3,       # PSUM cm tiles ([128,1024] fp32 = 2 banks each)
    junk_bufs=2,
    sc_bufs=2,        # per-group score tiles
    m_split=2,        # mat_M load DMA count (split along k)
    m_q="pool",       # mat_M ring: keep sync (ctx) and scalar (res g0) clear
    ctx_qs=("sp",),   # ring per ctxT group b-split
    res_split=4,      # a-dim splits per fused res group DMA (finer splits
                      # shorten the pipeline fill; pass-neutral)
    res_qs=("act", "pool"),  # rings for res splits, alternated per group
    tail_per_group=False,    # per-group tails cost ~3.5us/pass steady for
                             # ~0.5us of end latency (measured/model)
    cms_f16=True,     # downcast cm to fp16 in the ACT copy (DVE 2x mode)
    # timing-only probes (produce wrong outputs; never set in the real kernel)
    probe_no_compute=False,
    probe_nk=NK,
    probe_kc=KC,
)


def build_nc(repeat=1, internal_inputs=False, opts=None):
    """Build + compile the single-core Bass program (same program on all 8 cores).

    repeat>1 replays the steady-state compute loop; internal_inputs=True reads
    ctxT/res from internal DRAM scratch instead of ExternalInputs (both are
    timing aids only)."""
    nc = bacc.Bacc("TRN2", debug=False, enable_asserts=False, num_devices=NCORES)

    o = dict(DEFAULT_OPTS)
    if opts:
        o.update(opts)

    kind = "Internal" if internal_inputs else "ExternalInput"
    sfx = "_i" if internal_inputs else ""
    ctx_d = nc.dram_tensor("ctxB" + sfx, (NG, P, KC, BS // NG), F16, kind=kind)
    res_d = nc.dram_tensor("res_all" + sfx, (NG, P, GA, NK, E), F16, kind=kind)
    m_d = nc.dram_tensor("mat_M" + sfx, (E, E), F16, kind=kind)
    if internal_inputs:
        # keep one ExternalInput so the PJRT wrapper has something to bind
        nc.dram_tensor("mat_M", (E, E), F32, kind="ExternalInput")
    out_d = nc.dram_tensor("out", (BS,), F32, kind="ExternalOutput")

    with tile.TileContext(nc) as tc:
        _body(nc, tc, ctx_d.ap(), res_d.ap(), m_d.ap(), out_d.ap(),
              repeat=repeat, o=o)

    nc.compile()
    return nc


def _body(nc, tc, ctx_d, res_d, m_d, out_d, repeat=1, o=None):
    o = o or DEFAULT_OPTS
    # DRAM views. DMA pairs source/dest elements in flat AP order, so the
    # DRAM view dims must match the SBUF tile's dim order.
    # ctx_d is host-blocked [g, p, k, b_local]: each group's DMA is one
    # contiguous 1MB DRAM block with 8KB partition runs
    m_v = m_d.rearrange("(k p) e -> p k e", p=P)              # [p, k, e]

    ENG = {"sp": nc.sync, "act": nc.scalar, "pool": nc.gpsimd}
    GB = BS // NG  # ctxT columns per group

    with (
        tc.tile_pool(name="mpool", bufs=1) as mpool,
        tc.tile_pool(name="ctxp", bufs=o["ctx_bufs"]) as ctxp,
        tc.tile_pool(name="resp", bufs=o["res_bufs"]) as resp,
        tc.tile_pool(name="cmsb", bufs=o["cms_bufs"]) as cmsb,
        tc.tile_pool(name="junkp", bufs=o["junk_bufs"]) as junkp,
        tc.tile_pool(name="scp", bufs=o["sc_bufs"]) as scp,
        tc.tile_pool(name="tp", bufs=2) as tp,
        tc.tile_pool(name="smallp", bufs=1) as smallp,
        tc.tile_pool(name="pcm", bufs=o["pcm_bufs"], space="PSUM") as pcm,
    ):
        # mat_M resident: m_sb[p, k, :] = M[k*128 + p, :]  (fp16), split
        # along k so the first matmuls only wait for the first chunk
        m_sb = mpool.tile([P, KC, E], F16)
        ms = o["m_split"]
        for s in range(ms):
            k0, k1 = s * KC // ms, (s + 1) * KC // ms
            ENG[o["m_q"]].dma_start(m_sb[:, k0:k1, :], m_v[:, k0:k1, :])

        if o["tail_per_group"]:
            # per-group tails stage results here; ONE out DMA at pass end.
            # (An out DMA per group would sit in a DGE ring ahead of the
            # next group's loads and stall them at the sequencer wait.)
            outsb_full = smallp.tile([P, NT], F32)
        else:
            scores_full = smallp.tile([P, NT, NK], F32)

        for _rep in range(repeat):
            for g in range(NG):
                # per-group ctx tile [P, KC, GB]: 8KB/partition, so several
                # groups' ctx can be in flight alongside 3 res groups
                ctx_g = ctxp.tile([P, KC, GB], F16, tag="ctx")
                ENG[o["ctx_qs"][g % len(o["ctx_qs"])]].dma_start(
                    ctx_g[:], ctx_d[g]
                )
                # fused res group [P, GA, NK, E]: contiguous in DRAM,
                # split along a across the DGE rings
                res_t = resp.tile([P, GA, NK, E], F16, tag="res")
                rs = o["res_split"]
                nq = len(o["res_qs"])
                for s in range(rs):
                    a0, a1 = s * GA // rs, (s + 1) * GA // rs
                    eng = ENG[o["res_qs"][(g + s) % nq]]
                    eng.dma_start(res_t[:, a0:a1], res_d[g][:, a0:a1])

                if o["probe_no_compute"]:
                    for k in range(NK):
                        touch = junkp.tile([P, 16], F16, tag="junk")
                        nc.scalar.copy(touch[:], res_t[:, 0, k, :16])
                    touch = junkp.tile([P, 16], F16, tag="junk")
                    nc.scalar.copy(touch[:], ctx_g[:, 0, :16])
                    continue

                sc = None
                if o["tail_per_group"]:
                    sc = scp.tile([P, GA, NK], F32, tag="sc")

                for a in range(GA):
                    t = GA * g + a
                    # cm[128b, 1024e'] accumulated over 8 contraction chunks
                    cm = pcm.tile([P, E], F32, tag="cm")
                    for k in range(o["probe_kc"]):
                        for h in range(2):
                            nc.tensor.matmul(
                                cm[:, h * NHALF : (h + 1) * NHALF],
                                ctx_g[:, k, a * P : (a + 1) * P],
                                m_sb[:, k, h * NHALF : (h + 1) * NHALF],
                                start=(k == 0),
                                stop=(k == o["probe_kc"] - 1),
                            )

                    # PSUM -> SBUF with fp16 downcast: frees the PSUM bank and
                    # lets the DVE score ops run in 16-bit 2x mode
                    cdt = F16 if o["cms_f16"] else F32
                    cm_s = cmsb.tile([P, E], cdt, tag="cms")
                    nc.scalar.copy(cm_s[:], cm[:])

                    # scores[:, a, k] = sum_e' cm * res_k (fused mul+accum)
                    for k in range(o["probe_nk"]):
                        junk = junkp.tile([P, E], cdt, tag="junk")
                        acc = (sc[:, a, k : k + 1] if o["tail_per_group"]
                               else scores_full[:, t, k : k + 1])
                        nc.vector.scalar_tensor_tensor(
                            out=junk[:],
                            in0=cm_s[:],
                            scalar=1.0,
                            in1=res_t[:, a, k, :],
                            op0=mybir.AluOpType.mult,
                            op1=mybir.AluOpType.mult,
                            accum_out=acc,
                        )

                if o["tail_per_group"]:
                    _tail(nc, tp, sc, outsb_full[:, g * GA : (g + 1) * GA], GA)

        if o["probe_no_compute"]:
            outsb = smallp.tile([P, NT], F32)
            nc.vector.memset(outsb[:], 0.0)
            nc.sync.dma_start(out_d.rearrange("(t p) -> p t", p=P), outsb[:])
        elif o["tail_per_group"]:
            nc.sync.dma_start(out_d.rearrange("(t p) -> p t", p=P), outsb_full[:])
        else:
            outsb = smallp.tile([P, NT], F32)
            _tail(nc, smallp, scores_full, outsb[:], NT)
            nc.sync.dma_start(out_d.rearrange("(t p) -> p t", p=P), outsb[:])


def _tail(nc, pool, sc, out_sb, n):
    """log-softmax over [P, n, NK] score tile -> out_sb (SBUF AP, [P, n])."""
    mx = pool.tile([P, n], F32, tag="mx")
    nc.vector.tensor_reduce(
        out=mx[:], in_=sc[:], axis=mybir.AxisListType.X, op=mybir.AluOpType.max
    )
    d = pool.tile([P, n, NK], F32, tag="d")
    mx_b = mx[:, :, None].broadcast_to([P, n, NK])
    nc.vector.tensor_tensor(
        out=d[:], in0=sc[:], in1=mx_b, op=mybir.AluOpType.subtract
    )
    ex = pool.tile([P, n, NK], F32, tag="ex")
    nc.scalar.activation(ex[:], d[:], mybir.ActivationFunctionType.Exp)
    ssum = pool.tile([P, n], F32, tag="ssum")
    nc.vector.tensor_reduce(
        out=ssum[:], in_=ex[:], axis=mybir.AxisListType.X, op=mybir.AluOpType.add
    )
    lse = pool.tile([P, n], F32, tag="lse")
    nc.scalar.activation(lse[:], ssum[:], mybir.ActivationFunctionType.Ln)
    nc.vector.tensor_sub(out_sb, d[:, :, 0], lse[:])


_NC_CACHE = None


def _get_nc():
    global _NC_CACHE
    if _NC_CACHE is None:
        _NC_CACHE = build_nc()
    return _NC_CACHE


def make_in_maps(contexts, res_pos, res_neg1, res_neg2, res_neg3, res_neg4, mat_M):
    contexts = np.asarray(contexts, dtype=np.float32).astype(np.float16)
    ress = [
        np.asarray(r, dtype=np.float32).astype(np.float16)
        for r in (res_pos, res_neg1, res_neg2, res_neg3, res_neg4)
    ]
    mat_M16 = np.asarray(mat_M, dtype=np.float32).astype(np.float16)
    in_maps = []
    GB = BS // NG
    for c in range(NCORES):
        sl = slice(c * BS, (c + 1) * BS)
        # [E, BS] -> [KC, P, NG, GB] -> blocked [NG, P, KC, GB]
        ctxb = contexts[sl].T.reshape(KC, P, NG, GB).transpose(2, 1, 0, 3)
        m = {"ctxB": np.ascontiguousarray(ctxb), "mat_M": mat_M16}
        # [NK, BS, E] -> [NK, NG, GA, P, E] -> [NG, P, GA, NK, E]
        arr = np.stack([r[sl] for r in ress], axis=0)
        arr = arr.reshape(NK, NG, GA, P, E).transpose(1, 3, 2, 0, 4)
        m["res_all"] = np.ascontiguousarray(arr)
        in_maps.append(m)
    return in_maps


def kernel(contexts, res_pos, res_neg1, res_neg2, res_neg3, res_neg4, mat_M):
    nc = _get_nc()
    in_maps = make_in_maps(
        contexts, res_pos, res_neg1, res_neg2, res_neg3, res_neg4, mat_M
    )
    res = run_bass_kernel_spmd(nc, in_maps, core_ids=list(range(NCORES)))
    out = np.concatenate([res.results[c]["out"] for c in range(NCORES)])
    return out.astype(np.float32, copy=False)


# revision 54
# speedup vs baseline: 1.4666x; 1.4666x over previous
"""Trainium2 Bass kernel for ConvScoreSSREM loss (fp16-staged pipeline, v3).

Computes, for B=16384 rows (data-parallel, 2048 rows per NeuronCore x 8):
    cm        = contexts @ mat_M                    [B, E]
    scores_k  = sum_e cm[b,e] * res_k[b,e]          k in 0..4
    out[b]    = log_softmax(scores)[:, 0]

The kernel is HBM-streaming-bound.  Host-side staging transforms:
  - all streamed tensors (and mat_M) are cast to fp16 on the host before
    upload: HBM traffic halves (48MB -> 24MB per core per pass), and the
    DVE score reductions run in 16-bit 2x mode.  fp16 quantization gives
    3.6e-4 rel err on the output (gate is 2e-2); fp8 anywhere breaks the
    gate (measured 2.2e-2+ on the actual seeded inputs).
  - contexts is pre-transposed AND group-blocked on the host to
    [NG, P, KC, GB], so the PE consumes it as the stationary operand
    directly (no PE transposes) and each group's ctx DMA is one
    contiguous 1MB DRAM block with 8KB partition runs.
  - the five res tensors are interleaved into one [NG, P, GA, NK, E]
    DRAM tensor: each 512-row group loads as contiguous 1.25MB DMAs
    with 20KB partition runs.

v4-v6 target the single-pass critical path (what a one-shot kernel run
pays) and in-pass DMA lookahead on top of the v2 steady-state structure:
  - DGE rings are dedicated: ctx group-blocks on sync, res a-splits on
    scalar/gpsimd alternating, mat_M (two per-k-chunk DMAs) on gpsimd -
    the first matmuls/scores start after ~3MB lands, not ~6MB+.
  - ctx lives in per-group [P, KC, GB] tiles (8KB/partition) instead of
    one whole-pass 32KB tile; the freed SBUF holds a third res group in
    flight (res_bufs=3, ctx_bufs=5) for deeper DMA pipelining.
  - the log-softmax tail runs once at pass end into an SBUF staging tile
    with a single out DMA.  (A per-group out DMA is a trap: its
    semaphore wait holds the DGE ring FIFO at the sequencer and stalls
    every load queued behind it - measured 2x steady-state regression.)

Per-core steady state (2048 rows, E=1024): per 128-row tile, 16 matmuls
(stationary = ctxT chunk fp16, moving = mat_M 512-wide halves, one PSUM
bank each) accumulate cm[128,1024] fp32; ACT copies cm to SBUF with an
fp16 downcast; 5 DVE scalar_tensor_tensor ops (fp16 in, fp32 accum)
produce the scores.

Measured (axon, r41 wall differential): ~33-42us steady-state per pass
vs ~109us for the fp32 baseline; ~33us == 24MB / 716 GB/s, the
per-device HBM roofline.
"""

import numpy as np

import concourse.bacc as bacc
import concourse.mybir as mybir
import concourse.tile as tile
from concourse.bass_utils import run_bass_kernel_spmd

B = 16384
E = 1024
NCORES = 8
BS = B // NCORES  # 2048 rows per core
P = 128
NT = BS // P      # 16 row-tiles per core
KC = E // P       # 8 contraction chunks
NK = 5            # number of res tensors
NHALF = 512       # matmul moving free-dim (one PSUM bank of fp32)
GA = 4            # row-tiles per res DMA group (512 rows)
NG = NT // GA     # 4 groups per pass

F32 = mybir.dt.float32
F16 = mybir.dt.float16

RES_NAMES = ["res0", "res1", "res2", "res3", "res4"]


DEFAULT_OPTS = dict(
    res_bufs=3,       # whole-group [P,GA,NK,E] tiles in flight (40KB each)
    ctx_bufs=5,       # per-group [P,KC,GB] ctx tiles in flight (8KB each)
    cms_bufs=3,
    pcm_bufs=3,       # PSUM cm tiles ([128,1024] fp32 = 2 banks each)
    junk_bufs=2,
    sc_bufs=2,        # per-group score tiles
    m_split=2,        # mat_M load DMA count (split along k)
    m_q="pool",       # mat_M ring: keep sync (ctx) and scalar (res g0) clear
    ctx_qs=("sp",),   # ring per ctxT group b-split
    res_split=2,      # a-dim splits per fused res group DMA (fewer, bigger
                      # DMAs measure ~4us/pass faster in steady state)
    res_split0=2,     # group-0-only split override (fill tuning knob; the
                      # cost model's r1 deltas proved too chaotic to exploit)
    res_qs=("act", "pool"),  # rings for res splits, alternated per group
    tail_per_group=False,    # per-group tails cost ~3.5us/pass steady for
                             # ~0.5us of end latency (measured/model)
    cms_f16=True,     # downcast cm to fp16 in the ACT copy (DVE 2x mode)
    # timing-only probes (produce wrong outputs; never set in the real kernel)
    probe_no_compute=False,
    probe_nk=NK,
    probe_kc=KC,
)


def build_nc(repeat=1, internal_inputs=False, opts=None):
    """Build + compile the single-core Bass program (same program on all 8 cores).

    repeat>1 replays the steady-state compute loop; internal_inputs=True reads
    ctxT/res from internal DRAM scratch instead of ExternalInputs (both are
    timing aids only)."""
    nc = bacc.Bacc("TRN2", debug=False, enable_asserts=False, num_devices=NCORES)

    o = dict(DEFAULT_OPTS)
    if opts:
        o.update(opts)

    kind = "Internal" if internal_inputs else "ExternalInput"
    sfx = "_i" if internal_inputs else ""
    ctx_d = nc.dram_tensor("ctxB" + sfx, (NG, P, KC, BS // NG), F16, kind=kind)
    res_d = nc.dram_tensor("res_all" + sfx, (NG, P, GA, NK, E), F16, kind=kind)
    m_d = nc.dram_tensor("mat_M" + sfx, (E, E), F16, kind=kind)
    if internal_inputs:
        # keep one ExternalInput so the PJRT wrapper has something to bind
        nc.dram_tensor("mat_M", (E, E), F32, kind="ExternalInput")
    out_d = nc.dram_tensor("out", (BS,), F32, kind="ExternalOutput")

    with tile.TileContext(nc) as tc:
        _body(nc, tc, ctx_d.ap(), res_d.ap(), m_d.ap(), out_d.ap(),
              repeat=repeat, o=o)

    nc.compile()
    return nc


def _body(nc, tc, ctx_d, res_d, m_d, out_d, repeat=1, o=None):
    o = o or DEFAULT_OPTS
    # DRAM views. DMA pairs source/dest elements in flat AP order, so the
    # DRAM view dims must match the SBUF tile's dim order.
    # ctx_d is host-blocked [g, p, k, b_local]: each group's DMA is one
    # contiguous 1MB DRAM block with 8KB partition runs
    m_v = m_d.rearrange("(k p) e -> p k e", p=P)              # [p, k, e]

    ENG = {"sp": nc.sync, "act": nc.scalar, "pool": nc.gpsimd}
    GB = BS // NG  # ctxT columns per group

    with (
        tc.tile_pool(name="mpool", bufs=1) as mpool,
        tc.tile_pool(name="ctxp", bufs=o["ctx_bufs"]) as ctxp,
        tc.tile_pool(name="resp", bufs=o["res_bufs"]) as resp,
        tc.tile_pool(name="cmsb", bufs=o["cms_bufs"]) as cmsb,
        tc.tile_pool(name="junkp", bufs=o["junk_bufs"]) as junkp,
        tc.tile_pool(name="scp", bufs=o["sc_bufs"]) as scp,
        tc.tile_pool(name="tp", bufs=2) as tp,
        tc.tile_pool(name="smallp", bufs=1) as smallp,
        tc.tile_pool(name="pcm", bufs=o["pcm_bufs"], space="PSUM") as pcm,
    ):
        # mat_M resident: m_sb[p, k, :] = M[k*128 + p, :]  (fp16), split
        # along k so the first matmuls only wait for the first chunk
        m_sb = mpool.tile([P, KC, E], F16)
        ms = o["m_split"]
        for s in range(ms):
            k0, k1 = s * KC // ms, (s + 1) * KC // ms
            ENG[o["m_q"]].dma_start(m_sb[:, k0:k1, :], m_v[:, k0:k1, :])

        if o["tail_per_group"]:
            # per-group tails stage results here; ONE out DMA at pass end.
            # (An out DMA per group would sit in a DGE ring ahead of the
            # next group's loads and stall them at the sequencer wait.)
            outsb_full = smallp.tile([P, NT], F32)
        else:
            scores_full = smallp.tile([P, NT, NK], F32)

        for _rep in range(repeat):
            for g in range(NG):
                # per-group ctx tile [P, KC, GB]: 8KB/partition, so several
                # groups' ctx can be in flight alongside 3 res groups
                ctx_g = ctxp.tile([P, KC, GB], F16, tag="ctx")
                ENG[o["ctx_qs"][g % len(o["ctx_qs"])]].dma_start(
                    ctx_g[:], ctx_d[g]
                )
                # fused res group [P, GA, NK, E]: contiguous in DRAM,
                # split along a across the DGE rings
                res_t = resp.tile([P, GA, NK, E], F16, tag="res")
                rs = o["res_split0"] if (_rep == 0 and g == 0) else o["res_split"]
                nq = len(o["res_qs"])
                for s in range(rs):
                    a0, a1 = s * GA // rs, (s + 1) * GA // rs
                    eng = ENG[o["res_qs"][(g + s) % nq]]
                    eng.dma_start(res_t[:, a0:a1], res_d[g][:, a0:a1])

                if o["probe_no_compute"]:
                    for k in range(NK):
                        touch = junkp.tile([P, 16], F16, tag="junk")
                        nc.scalar.copy(touch[:], res_t[:, 0, k, :16])
                    touch = junkp.tile([P, 16], F16, tag="junk")
                    nc.scalar.copy(touch[:], ctx_g[:, 0, :16])
                    continue

                sc = None
                if o["tail_per_group"]:
                    sc = scp.tile([P, GA, NK], F32, tag="sc")

                for a in range(GA):
                    t = GA * g + a
                    # cm[128b, 1024e'] accumulated over 8 contraction chunks
                    cm = pcm.tile([P, E], F32, tag="cm")
                    for k in range(o["probe_kc"]):
                        for h in range(2):
                            nc.tensor.matmul(
                                cm[:, h * NHALF : (h + 1) * NHALF],
                                ctx_g[:, k, a * P : (a + 1) * P],
                                m_sb[:, k, h * NHALF : (h + 1) * NHALF],
                                start=(k == 0),
                                stop=(k == o["probe_kc"] - 1),
                            )

                    # PSUM -> SBUF with fp16 downcast: frees the PSUM bank and
                    # lets the DVE score ops run in 16-bit 2x mode
                    cdt = F16 if o["cms_f16"] else F32
                    cm_s = cmsb.tile([P, E], cdt, tag="cms")
                    nc.scalar.copy(cm_s[:], cm[:])

                    # scores[:, a, k] = sum_e' cm * res_k (fused mul+accum)
                    for k in range(o["probe_nk"]):
                        junk = junkp.tile([P, E], cdt, tag="junk")
                        acc = (sc[:, a, k : k + 1] if o["tail_per_group"]
                               else scores_full[:, t, k : k + 1])
                        nc.vector.scalar_tensor_tensor(
                            out=junk[:],
                            in0=cm_s[:],
                            scalar=1.0,
                            in1=res_t[:, a, k, :],
                            op0=mybir.AluOpType.mult,
                            op1=mybir.AluOpType.mult,
                            accum_out=acc,
                        )

                if o["tail_per_group"]:
                    _tail(nc, tp, sc, outsb_full[:, g * GA : (g + 1) * GA], GA)

        if o["probe_no_compute"]:
            outsb = smallp.tile([P, NT], F32)
            nc.vector.memset(outsb[:], 0.0)
            nc.sync.dma_start(out_d.rearrange("(t p) -> p t", p=P), outsb[:])
        elif o["tail_per_group"]:
            nc.sync.dma_start(out_d.rearrange("(t p) -> p t", p=P), outsb_full[:])
        else:
            outsb = smallp.tile([P, NT], F32)
            _tail(nc, smallp, scores_full, outsb[:], NT)
            nc.sync.dma_start(out_d.rearrange("(t p) -> p t", p=P), outsb[:])


def _tail(nc, pool, sc, out_sb, n):
    """log-softmax over [P, n, NK] score tile -> out_sb (SBUF AP, [P, n])."""
    mx = pool.tile([P, n], F32, tag="mx")
    nc.vector.tensor_reduce(
        out=mx[:], in_=sc[:], axis=mybir.AxisListType.X, op=mybir.AluOpType.max
    )
    d = pool.tile([P, n, NK], F32, tag="d")
    mx_b = mx[:, :, None].broadcast_to([P, n, NK])
    nc.vector.tensor_tensor(
        out=d[:], in0=sc[:], in1=mx_b, op=mybir.AluOpType.subtract
    )
    ex = pool.tile([P, n, NK], F32, tag="ex")
    nc.scalar.activation(ex[:], d[:], mybir.ActivationFunctionType.Exp)
    ssum = pool.tile([P, n], F32, tag="ssum")
    nc.vector.tensor_reduce(
        out=ssum[:], in_=ex[:], axis=mybir.AxisListType.X, op=mybir.AluOpType.add
    )
    lse = pool.tile([P, n], F32, tag="lse")
    nc.scalar.activation(lse[:], ssum[:], mybir.ActivationFunctionType.Ln)
    nc.vector.tensor_sub(out_sb, d[:, :, 0], lse[:])


_NC_CACHE = None


def _get_nc():
    global _NC_CACHE
    if _NC_CACHE is None:
        _NC_CACHE = build_nc()
    return _NC_CACHE


def make_in_maps(contexts, res_pos, res_neg1, res_neg2, res_neg3, res_neg4, mat_M):
    contexts = np.asarray(contexts, dtype=np.float32).astype(np.float16)
    ress = [
        np.asarray(r, dtype=np.float32).astype(np.float16)
        for r in (res_pos, res_neg1, res_neg2, res_neg3, res_neg4)
    ]
    mat_M16 = np.asarray(mat_M, dtype=np.float32).astype(np.float16)
    in_maps = []
    GB = BS // NG
    for c in range(NCORES):
        sl = slice(c * BS, (c + 1) * BS)
        # [E, BS] -> [KC, P, NG, GB] -> blocked [NG, P, KC, GB]
        ctxb = contexts[sl].T.reshape(KC, P, NG, GB).transpose(2, 1, 0, 3)
        m = {"ctxB": np.ascontiguousarray(ctxb), "mat_M": mat_M16}
        # [NK, BS, E] -> [NK, NG, GA, P, E] -> [NG, P, GA, NK, E]
        arr = np.stack([r[sl] for r in ress], axis=0)
        arr = arr.reshape(NK, NG, GA, P, E).transpose(1, 3, 2, 0, 4)
        m["res_all"] = np.ascontiguousarray(arr)
        in_maps.append(m)
    return in_maps


def kernel(contexts, res_pos, res_neg1, res_neg2, res_neg3, res_neg4, mat_M):
    nc = _get_nc()
    in_maps = make_in_maps(
        contexts, res_pos, res_neg1, res_neg2, res_neg3, res_neg4, mat_M
    )
    res = run_bass_kernel_spmd(nc, in_maps, core_ids=list(range(NCORES)))
    out = np.concatenate([res.results[c]["out"] for c in range(NCORES)])
    return out.astype(np.float32, copy=False)


# revision 55
# speedup vs baseline: 1.5997x; 1.0908x over previous
"""Trainium2 Bass kernel for ConvScoreSSREM loss (fp16-staged pipeline, v3).

Computes, for B=16384 rows (data-parallel, 2048 rows per NeuronCore x 8):
    cm        = contexts @ mat_M                    [B, E]
    scores_k  = sum_e cm[b,e] * res_k[b,e]          k in 0..4
    out[b]    = log_softmax(scores)[:, 0]

The kernel is HBM-streaming-bound.  Host-side staging transforms:
  - all streamed tensors (and mat_M) are cast to fp16 on the host before
    upload: HBM traffic halves (48MB -> 24MB per core per pass), and the
    DVE score reductions run in 16-bit 2x mode.  fp16 quantization gives
    3.6e-4 rel err on the output (gate is 2e-2); fp8 anywhere breaks the
    gate (measured 2.2e-2+ on the actual seeded inputs).
  - contexts is pre-transposed AND group-blocked on the host to
    [NG, P, KC, GB], so the PE consumes it as the stationary operand
    directly (no PE transposes) and each group's ctx DMA is one
    contiguous 1MB DRAM block with 8KB partition runs.
  - the five res tensors are interleaved into one [NG, P, GA, NK, E]
    DRAM tensor: each 512-row group loads as contiguous 1.25MB DMAs
    with 20KB partition runs.

v4-v6 target the single-pass critical path (what a one-shot kernel run
pays) and in-pass DMA lookahead on top of the v2 steady-state structure:
  - DGE rings are dedicated: ctx group-blocks on sync, res a-splits on
    scalar/gpsimd alternating, mat_M (two per-k-chunk DMAs) on gpsimd -
    the first matmuls/scores start after ~3MB lands, not ~6MB+.
  - ctx lives in per-group [P, KC, GB] tiles (8KB/partition) instead of
    one whole-pass 32KB tile; the freed SBUF holds a third res group in
    flight (res_bufs=3, ctx_bufs=5) for deeper DMA pipelining.
  - the log-softmax tail runs once at pass end into an SBUF staging tile
    with a single out DMA.  (A per-group out DMA is a trap: its
    semaphore wait holds the DGE ring FIFO at the sequencer and stalls
    every load queued behind it - measured 2x steady-state regression.)

Per-core steady state (2048 rows, E=1024): per 128-row tile, 16 matmuls
(stationary = ctxT chunk fp16, moving = mat_M 512-wide halves, one PSUM
bank each) accumulate cm[128,1024] fp32; ACT copies cm to SBUF with an
fp16 downcast; 5 DVE scalar_tensor_tensor ops (fp16 in, fp32 accum)
produce the scores.

Measured (axon, r41 wall differential): ~33-42us steady-state per pass
vs ~109us for the fp32 baseline; ~33us == 24MB / 716 GB/s, the
per-device HBM roofline.
"""

import numpy as np

import concourse.bacc as bacc
import concourse.mybir as mybir
import concourse.tile as tile
from concourse.bass_utils import run_bass_kernel_spmd

B = 16384
E = 1024
NCORES = 8
BS = B // NCORES  # 2048 rows per core
P = 128
NT = BS // P      # 16 row-tiles per core
KC = E // P       # 8 contraction chunks
NK = 5            # number of res tensors
NHALF = 512       # matmul moving free-dim (one PSUM bank of fp32)
GA = 4            # row-tiles per res DMA group (512 rows)
NG = NT // GA     # 4 groups per pass

F32 = mybir.dt.float32
F16 = mybir.dt.float16

RES_NAMES = ["res0", "res1", "res2", "res3", "res4"]


DEFAULT_OPTS = dict(
    res_bufs=3,       # whole-group [P,GA,NK,E] tiles in flight (40KB each)
    ctx_bufs=5,       # per-group [P,KC,GB] ctx tiles in flight (8KB each)
    cms_bufs=3,
    pcm_bufs=3,       # PSUM cm tiles ([128,1024] fp32 = 2 banks each)
    junk_bufs=2,
    sc_bufs=2,        # per-group score tiles
    m_split=2,        # mat_M load DMA count (split along k)
    m_q="pool",       # mat_M ring: keep sync (ctx) and scalar (res g0) clear
    ctx_qs=("sp",),   # ring per ctxT group b-split
    res_split=2,      # a-dim splits per fused res group DMA (fewer, bigger
                      # DMAs measure ~4us/pass faster in steady state)
    res_split0=2,     # group-0-only split override (fill tuning knob; the
                      # cost model's r1 deltas proved too chaotic to exploit)
    res_qs=("act",),  # all res on the scalar HWDGE ring: keeping the 20MB
                      # stream off SWDGE measured ~3-7us/pass faster in two
                      # paired A/Bs (Q7 descriptor-gen + ring AXI contention)
    tail_per_group=False,    # per-group tails cost ~3.5us/pass steady for
                             # ~0.5us of end latency (measured/model)
    cms_f16=True,     # downcast cm to fp16 in the ACT copy (DVE 2x mode)
    # timing-only probes (produce wrong outputs; never set in the real kernel)
    probe_no_compute=False,
    probe_nk=NK,
    probe_kc=KC,
)


def build_nc(repeat=1, internal_inputs=False, opts=None):
    """Build + compile the single-core Bass program (same program on all 8 cores).

    repeat>1 replays the steady-state compute loop; internal_inputs=True reads
    ctxT/res from internal DRAM scratch instead of ExternalInputs (both are
    timing aids only)."""
    nc = bacc.Bacc("TRN2", debug=False, enable_asserts=False, num_devices=NCORES)

    o = dict(DEFAULT_OPTS)
    if opts:
        o.update(opts)

    kind = "Internal" if internal_inputs else "ExternalInput"
    sfx = "_i" if internal_inputs else ""
    ctx_d = nc.dram_tensor("ctxB" + sfx, (NG, P, KC, BS // NG), F16, kind=kind)
    res_d = nc.dram_tensor("res_all" + sfx, (NG, P, GA, NK, E), F16, kind=kind)
    m_d = nc.dram_tensor("mat_M" + sfx, (E, E), F16, kind=kind)
    if internal_inputs:
        # keep one ExternalInput so the PJRT wrapper has something to bind
        nc.dram_tensor("mat_M", (E, E), F32, kind="ExternalInput")
    out_d = nc.dram_tensor("out", (BS,), F32, kind="ExternalOutput")

    with tile.TileContext(nc) as tc:
        _body(nc, tc, ctx_d.ap(), res_d.ap(), m_d.ap(), out_d.ap(),
              repeat=repeat, o=o)

    nc.compile()
    return nc


def _body(nc, tc, ctx_d, res_d, m_d, out_d, repeat=1, o=None):
    o = o or DEFAULT_OPTS
    # DRAM views. DMA pairs source/dest elements in flat AP order, so the
    # DRAM view dims must match the SBUF tile's dim order.
    # ctx_d is host-blocked [g, p, k, b_local]: each group's DMA is one
    # contiguous 1MB DRAM block with 8KB partition runs
    m_v = m_d.rearrange("(k p) e -> p k e", p=P)              # [p, k, e]

    ENG = {"sp": nc.sync, "act": nc.scalar, "pool": nc.gpsimd}
    GB = BS // NG  # ctxT columns per group

    with (
        tc.tile_pool(name="mpool", bufs=1) as mpool,
        tc.tile_pool(name="ctxp", bufs=o["ctx_bufs"]) as ctxp,
        tc.tile_pool(name="resp", bufs=o["res_bufs"]) as resp,
        tc.tile_pool(name="cmsb", bufs=o["cms_bufs"]) as cmsb,
        tc.tile_pool(name="junkp", bufs=o["junk_bufs"]) as junkp,
        tc.tile_pool(name="scp", bufs=o["sc_bufs"]) as scp,
        tc.tile_pool(name="tp", bufs=2) as tp,
        tc.tile_pool(name="smallp", bufs=1) as smallp,
        tc.tile_pool(name="pcm", bufs=o["pcm_bufs"], space="PSUM") as pcm,
    ):
        # mat_M resident: m_sb[p, k, :] = M[k*128 + p, :]  (fp16), split
        # along k so the first matmuls only wait for the first chunk
        m_sb = mpool.tile([P, KC, E], F16)
        ms = o["m_split"]
        for s in range(ms):
            k0, k1 = s * KC // ms, (s + 1) * KC // ms
            ENG[o["m_q"]].dma_start(m_sb[:, k0:k1, :], m_v[:, k0:k1, :])

        if o["tail_per_group"]:
            # per-group tails stage results here; ONE out DMA at pass end.
            # (An out DMA per group would sit in a DGE ring ahead of the
            # next group's loads and stall them at the sequencer wait.)
            outsb_full = smallp.tile([P, NT], F32)
        else:
            scores_full = smallp.tile([P, NT, NK], F32)

        for _rep in range(repeat):
            for g in range(NG):
                # per-group ctx tile [P, KC, GB]: 8KB/partition, so several
                # groups' ctx can be in flight alongside 3 res groups
                ctx_g = ctxp.tile([P, KC, GB], F16, tag="ctx")
                ENG[o["ctx_qs"][g % len(o["ctx_qs"])]].dma_start(
                    ctx_g[:], ctx_d[g]
                )
                # fused res group [P, GA, NK, E]: contiguous in DRAM,
                # split along a across the DGE rings
                res_t = resp.tile([P, GA, NK, E], F16, tag="res")
                rs = o["res_split0"] if (_rep == 0 and g == 0) else o["res_split"]
                nq = len(o["res_qs"])
                for s in range(rs):
                    a0, a1 = s * GA // rs, (s + 1) * GA // rs
                    eng = ENG[o["res_qs"][(g + s) % nq]]
                    eng.dma_start(res_t[:, a0:a1], res_d[g][:, a0:a1])

                if o["probe_no_compute"]:
                    for k in range(NK):
                        touch = junkp.tile([P, 16], F16, tag="junk")
                        nc.scalar.copy(touch[:], res_t[:, 0, k, :16])
                    touch = junkp.tile([P, 16], F16, tag="junk")
                    nc.scalar.copy(touch[:], ctx_g[:, 0, :16])
                    continue

                sc = None
                if o["tail_per_group"]:
                    sc = scp.tile([P, GA, NK], F32, tag="sc")

                for a in range(GA):
                    t = GA * g + a
                    # cm[128b, 1024e'] accumulated over 8 contraction chunks
                    cm = pcm.tile([P, E], F32, tag="cm")
                    for k in range(o["probe_kc"]):
                        for h in range(2):
                            nc.tensor.matmul(
                                cm[:, h * NHALF : (h + 1) * NHALF],
                                ctx_g[:, k, a * P : (a + 1) * P],
                                m_sb[:, k, h * NHALF : (h + 1) * NHALF],
                                start=(k == 0),
                                stop=(k == o["probe_kc"] - 1),
                            )

                    # PSUM -> SBUF with fp16 downcast: frees the PSUM bank and
                    # lets the DVE score ops run in 16-bit 2x mode
                    cdt = F16 if o["cms_f16"] else F32
                    cm_s = cmsb.tile([P, E], cdt, tag="cms")
                    nc.scalar.copy(cm_s[:], cm[:])

                    # scores[:, a, k] = sum_e' cm * res_k (fused mul+accum)
                    for k in range(o["probe_nk"]):
                        junk = junkp.tile([P, E], cdt, tag="junk")
                        acc = (sc[:, a, k : k + 1] if o["tail_per_group"]
                               else scores_full[:, t, k : k + 1])
                        nc.vector.scalar_tensor_tensor(
                            out=junk[:],
                            in0=cm_s[:],
                            scalar=1.0,
                            in1=res_t[:, a, k, :],
                            op0=mybir.AluOpType.mult,
                            op1=mybir.AluOpType.mult,
                            accum_out=acc,
                        )

                if o["tail_per_group"]:
                    _tail(nc, tp, sc, outsb_full[:, g * GA : (g + 1) * GA], GA)

        if o["probe_no_compute"]:
            outsb = smallp.tile([P, NT], F32)
            nc.vector.memset(outsb[:], 0.0)
            nc.sync.dma_start(out_d.rearrange("(t p) -> p t", p=P), outsb[:])
        elif o["tail_per_group"]:
            nc.sync.dma_start(out_d.rearrange("(t p) -> p t", p=P), outsb_full[:])
        else:
            outsb = smallp.tile([P, NT], F32)
            _tail(nc, smallp, scores_full, outsb[:], NT)
            nc.sync.dma_start(out_d.rearrange("(t p) -> p t", p=P), outsb[:])


def _tail(nc, pool, sc, out_sb, n):
    """log-softmax over [P, n, NK] score tile -> out_sb (SBUF AP, [P, n])."""
    mx = pool.tile([P, n], F32, tag="mx")
    nc.vector.tensor_reduce(
        out=mx[:], in_=sc[:], axis=mybir.AxisListType.X, op=mybir.AluOpType.max
    )
    d = pool.tile([P, n, NK], F32, tag="d")
    mx_b = mx[:, :, None].broadcast_to([P, n, NK])
    nc.vector.tensor_tensor(
        out=d[:], in0=sc[:], in1=mx_b, op=mybir.AluOpType.subtract
    )
    ex = pool.tile([P, n, NK], F32, tag="ex")
    nc.scalar.activation(ex[:], d[:], mybir.ActivationFunctionType.Exp)
    ssum = pool.tile([P, n], F32, tag="ssum")
    nc.vector.tensor_reduce(
        out=ssum[:], in_=ex[:], axis=mybir.AxisListType.X, op=mybir.AluOpType.add
    )
    lse = pool.tile([P, n], F32, tag="lse")
    nc.scalar.activation(lse[:], ssum[:], mybir.ActivationFunctionType.Ln)
    nc.vector.tensor_sub(out_sb, d[:, :, 0], lse[:])


_NC_CACHE = None


def _get_nc():
    global _NC_CACHE
    if _NC_CACHE is None:
        _NC_CACHE = build_nc()
    return _NC_CACHE


def make_in_maps(contexts, res_pos, res_neg1, res_neg2, res_neg3, res_neg4, mat_M):
    contexts = np.asarray(contexts, dtype=np.float32).astype(np.float16)
    ress = [
        np.asarray(r, dtype=np.float32).astype(np.float16)
        for r in (res_pos, res_neg1, res_neg2, res_neg3, res_neg4)
    ]
    mat_M16 = np.asarray(mat_M, dtype=np.float32).astype(np.float16)
    in_maps = []
    GB = BS // NG
    for c in range(NCORES):
        sl = slice(c * BS, (c + 1) * BS)
        # [E, BS] -> [KC, P, NG, GB] -> blocked [NG, P, KC, GB]
        ctxb = contexts[sl].T.reshape(KC, P, NG, GB).transpose(2, 1, 0, 3)
        m = {"ctxB": np.ascontiguousarray(ctxb), "mat_M": mat_M16}
        # [NK, BS, E] -> [NK, NG, GA, P, E] -> [NG, P, GA, NK, E]
        arr = np.stack([r[sl] for r in ress], axis=0)
        arr = arr.reshape(NK, NG, GA, P, E).transpose(1, 3, 2, 0, 4)
        m["res_all"] = np.ascontiguousarray(arr)
        in_maps.append(m)
    return in_maps


def kernel(contexts, res_pos, res_neg1, res_neg2, res_neg3, res_neg4, mat_M):
    nc = _get_nc()
    in_maps = make_in_maps(
        contexts, res_pos, res_neg1, res_neg2, res_neg3, res_neg4, mat_M
    )
    res = run_bass_kernel_spmd(nc, in_maps, core_ids=list(range(NCORES)))
    out = np.concatenate([res.results[c]["out"] for c in range(NCORES)])
    return out.astype(np.float32, copy=False)
